# revision 8
# baseline (speedup 1.0000x reference)
"""DiffusionGraphConv on 8 Trainium2 NeuronCores (Bass/Tile).

Reference computation (see problem):
  x0 = rearrange(x, 'b n d -> n (b d)')            # (N, F), F = B*D
  for each graph (rows, cols, vals):
      x1 = A @ x0        (A sparse: y[r] += v * x0[c])
      x2 = 2*A @ x1 - x0
  xs = [x0, x1_g0, x2_g0, x1_g1, x2_g1]            # Km = 5
  out[(b,n), :] = concat_k feat @ weight + biases  # weight (D*Km, O)

Distribution: 1D row-partition of N across 8 cores (each core owns the
edges whose destination row lives locally).  Per destination tile of 128
rows, edges are gathered 128 at a time with gpsimd.dma_gather (bf16,
1KB/row), a one-hot scatter matrix H (built on-chip with a single fused
tensor_scalar: H = (iota == dloc) * val) maps edges to destination rows
via a TensorE matmul accumulated in PSUM.  x1 slices are exchanged with
an AllGather collective; the final dense matmul runs over DMA-transposed
(dma_gather transpose=True) Xk tiles so the D-contraction lands on the
partition axis with zero on-chip transposes.

Host-side work is limited to sparse-format conversion (sorting edge
lists, padding, packing index tensors) and layout marshalling of inputs
and outputs; every FLOP that touches x runs on the NeuronCores.
"""

import math
import sys

for _p in ("/opt/trn_rl_repo",):
    if _p not in sys.path:
        sys.path.append(_p)

import numpy as np
import ml_dtypes
from contextlib import ExitStack

import concourse.bass as bass
import concourse.bacc as bacc
import concourse.tile as tile
from concourse import mybir
from concourse.library_config import mlp

BF16 = mybir.dt.bfloat16
F32 = mybir.dt.float32
I16 = mybir.dt.int16


def _pack_gather_idx(blocks):
    """blocks: (..., NCALL) int array -> [128, prod(...) * NCALL//16] int16.

    dma_gather expects index i of a call's block at [i % 16, i // 16],
    replicated 8x across the 128 partitions; call blocks are concatenated
    along the free axis.
    """
    lead = blocks.shape[:-1]
    ncall = blocks.shape[-1]
    a = blocks.reshape(-1, ncall // 16, 16)          # [nblk, s, p]
    a = np.transpose(a, (2, 0, 1)).reshape(16, -1)   # [16, nblk * ncall//16]
    return np.ascontiguousarray(np.tile(a, (8, 1)).astype(np.int16))


def _prep_graph(rows, cols, vals, N, ncores, ntiles):
    """Sort/pad one graph's edges into per-core gather+onehot tensors.

    Returns (idx_packed[core], dloc_cols[core], vals_cols[core], nch) with
    nch resolved globally later; here returns raw grouped arrays.
    """
    nloc = N // ncores
    half = N // 2
    rows = np.asarray(rows).astype(np.int64)
    cols = np.asarray(cols).astype(np.int64)
    vals = np.asarray(vals).astype(np.float32)

    core = rows // nloc
    rl = rows - core * nloc
    tl = rl >> 7
    dloc = rl & 127
    h = (cols >= half).astype(np.int64)
    srcl = cols - h * half

    gkey = (core * ntiles + tl) * 2 + h
    order = np.argsort(gkey, kind="stable")
    gs = gkey[order]
    ngroups = ncores * ntiles * 2
    counts = np.bincount(gkey, minlength=ngroups)
    return dict(
        srcl=srcl[order], dloc=dloc[order], vals=vals[order],
        gs=gs, counts=counts, ngroups=ngroups,
    )


def _pad_graph(g, nch, ncores, ntiles):
    ncall = nch * 128
    starts = np.zeros(g["ngroups"] + 1, np.int64)
    np.cumsum(g["counts"], out=starts[1:])
    rank = np.arange(len(g["gs"])) - np.repeat(starts[:-1], g["counts"])
    slot = g["gs"] * ncall + rank

    idx_pad = np.zeros(g["ngroups"] * ncall, np.int64)
    dloc_pad = np.full(g["ngroups"] * ncall, -1.0, np.float32)
    vals_pad = np.zeros(g["ngroups"] * ncall, np.float32)
    idx_pad[slot] = g["srcl"]
    dloc_pad[slot] = g["dloc"]
    vals_pad[slot] = g["vals"]

    idx_pad = idx_pad.reshape(ncores, ntiles, 2, ncall)
    dloc_pad = dloc_pad.reshape(ncores, ntiles, 2, nch, 128)
    vals_pad = vals_pad.reshape(ncores, ntiles, 2, nch, 128)

    idx_t, dloc_t, vals_t = [], [], []
    for m in range(ncores):
        idx_t.append(_pack_gather_idx(idx_pad[m]))
        dloc_t.append(np.ascontiguousarray(
            np.transpose(dloc_pad[m], (3, 0, 1, 2)).reshape(128, -1)))
        vals_t.append(np.ascontiguousarray(
            np.transpose(vals_pad[m], (3, 0, 1, 2)).reshape(128, -1)))
    return idx_t, dloc_t, vals_t


def _build_program(N, B, D, nch, ncores):
    """Emit the SPMD Bass program (identical for all cores)."""
    F = B * D
    Km = 5
    nloc = N // ncores
    ntiles = math.ceil(nloc / 128)
    nlocp = ntiles * 128
    half = N // 2
    ncall = nch * 128
    fch = F // 128          # feature chunks of 128 (4)
    idx_cols = ntiles * 2 * (ncall // 16)
    col_cols = ntiles * 2 * nch

    nc = bacc.Bacc("TRN2", target_bir_lowering=False, num_devices=ncores)

    x0h = [nc.dram_tensor(f"x0h{i}", [half, F], BF16, kind="ExternalInput")
           for i in range(2)]
    x0sb = nc.dram_tensor("x0sb", [nlocp, F], BF16, kind="ExternalInput")
    x0sf = nc.dram_tensor("x0sf", [nloc, F], F32, kind="ExternalInput")
    idx_d = [nc.dram_tensor(f"idx{g}", [128, idx_cols], I16, kind="ExternalInput")
             for g in range(2)]
    dl_d = [nc.dram_tensor(f"dl{g}", [128, col_cols], F32, kind="ExternalInput")
            for g in range(2)]
    vl_d = [nc.dram_tensor(f"vl{g}", [128, col_cols], F32, kind="ExternalInput")
            for g in range(2)]
    wblk_d = nc.dram_tensor("wblk", [128, Km * 128], BF16, kind="ExternalInput")
    bias_d = nc.dram_tensor("bias", [128, 1], F32, kind="ExternalInput")
    seq_d = nc.dram_tensor("seqidx", [128, ntiles * 8], I16, kind="ExternalInput")
    outT = nc.dram_tensor("outT", [fch, 128, nlocp], F32, kind="ExternalOutput")

    with tile.TileContext(nc) as tc:
        with ExitStack() as ctx:
            const = ctx.enter_context(tc.tile_pool(name="const", bufs=1))
            dram = ctx.enter_context(tc.tile_pool(name="dram", bufs=1, space="DRAM"))
            sg = ctx.enter_context(tc.tile_pool(name="sg", bufs=2))
            hp = ctx.enter_context(tc.tile_pool(name="hp", bufs=4))
            ev = ctx.enter_context(tc.tile_pool(name="ev", bufs=3))
            x0p = ctx.enter_context(tc.tile_pool(name="x0p", bufs=2))
            xkp = ctx.enter_context(tc.tile_pool(name="xkp", bufs=10))
            outp = ctx.enter_context(tc.tile_pool(name="outp", bufs=4))
            psum = ctx.enter_context(tc.tile_pool(name="psum", bufs=2, space="PSUM"))
            psumf = ctx.enter_context(tc.tile_pool(name="psumf", bufs=2, space="PSUM"))

            nc.gpsimd.load_library(mlp)

            # ---- static tiles --------------------------------------------
            idx_t, dl_t, vl_t = [], [], []
            for g in range(2):
                it = const.tile([128, idx_cols], I16, tag=f"idx{g}", name=f"idx{g}t")
                nc.sync.dma_start(it[:], idx_d[g][:])
                idx_t.append(it)
                dt_ = const.tile([128, col_cols], F32, tag=f"dl{g}", name=f"dl{g}t")
                nc.sync.dma_start(dt_[:], dl_d[g][:])
                dl_t.append(dt_)
                vt = const.tile([128, col_cols], F32, tag=f"vl{g}", name=f"vl{g}t")
                nc.sync.dma_start(vt[:], vl_d[g][:])
                vl_t.append(vt)
            iota_t = const.tile([128, 128], F32, tag="iota")
            nc.gpsimd.iota(iota_t[:], pattern=[[1, 128]], base=0,
                           channel_multiplier=0,
                           allow_small_or_imprecise_dtypes=True)
            wblk_t = const.tile([128, Km * 128], BF16, tag="wblk")
            nc.sync.dma_start(wblk_t[:], wblk_d[:])
            bias_t = const.tile([128, 1], F32, tag="bias")
            nc.sync.dma_start(bias_t[:], bias_d[:])
            seq_t = const.tile([128, ntiles * 8], I16, tag="seq")
            nc.sync.dma_start(seq_t[:], seq_d[:])

            # ---- internal DRAM -------------------------------------------
            x1s = [dram.tile([nlocp, F], BF16, tag=f"x1s{g}", name=f"x1s{g}") for g in range(2)]
            x1f = [dram.tile([N, F], BF16, tag=f"x1f{g}", name=f"x1f{g}",
                             addr_space="Shared") for g in range(2)]
            x2s = [dram.tile([nlocp, F], BF16, tag=f"x2s{g}", name=f"x2s{g}") for g in range(2)]

            def spmm_pass(g, src_half_aps, out_slice, subtract_x0):
                for t in range(ntiles):
                    s_tiles = []
                    for h in range(2):
                        st = sg.tile([128, nch, F], BF16, tag=f"s{h}", name=f"st{h}")
                        blk = (t * 2 + h) * (ncall // 16)
                        # SWDGE descriptor-ring carveout limits one call to
                        # ~1024 indices (64-desc/ring); split into sub-calls.
                        for j0 in range(0, nch, 8):
                            jn = min(8, nch - j0)
                            nc.gpsimd.dma_gather(
                                st[:, j0:j0 + jn, :], src_half_aps[h],
                                idx_t[g][:, blk + j0 * 8: blk + (j0 + jn) * 8],
                                jn * 128, jn * 128, F)
                        s_tiles.append(st)
                    acc = psum.tile([128, F], F32, tag="acc", name="acc")
                    for h in range(2):
                        for j in range(nch):
                            col = (t * 2 + h) * nch + j
                            ht = hp.tile([128, 128], BF16, tag="h", name="ht")
                            nc.vector.tensor_scalar(
                                ht[:], iota_t[:],
                                dl_t[g][:, col: col + 1],
                                vl_t[g][:, col: col + 1],
                                mybir.AluOpType.is_equal, mybir.AluOpType.mult)
                            nc.tensor.matmul(
                                acc[:], ht[:], s_tiles[h][:, j, :],
                                start=(h == 0 and j == 0),
                                stop=(h == 1 and j == nch - 1))
                    yb = ev.tile([128, F], BF16, tag="yb", name="yb")
                    if subtract_x0:
                        nrow = min(128, nloc - t * 128)
                        x0t = x0p.tile([128, F], F32, tag="x0t", name="x0t")
                        nc.sync.dma_start(x0t[:nrow, :],
                                          x0sf[t * 128: t * 128 + nrow, :])
                        # x2 = 2*psum - x0   (one fused DVE op)
                        nc.vector.scalar_tensor_tensor(
                            yb[:], acc[:], 2.0, x0t[:],
                            mybir.AluOpType.mult, mybir.AluOpType.subtract)
                    else:
                        nc.vector.tensor_copy(yb[:], acc[:])
                    nc.sync.dma_start(out_slice[t * 128:(t + 1) * 128, :], yb[:])

            # ---- passes: x1 = A x0 (both graphs), with AG overlap ---------
            x0_halves = [x0h[0][:, :], x0h[1][:, :]]
            for g in range(2):
                spmm_pass(g, x0_halves, x1s[g], subtract_x0=False)
                nc.gpsimd.collective_compute(
                    "AllGather", mybir.AluOpType.bypass,
                    replica_groups=[list(range(ncores))],
                    ins=[x1s[g][0:nloc, :].opt()],
                    outs=[x1f[g][:, :].opt()])

            # ---- passes: x2 = 2 A x1 - x0 --------------------------------
            for g in range(2):
                halves = [x1f[g][0:half, :], x1f[g][half:2 * half, :]]
                spmm_pass(g, halves, x2s[g], subtract_x0=True)

            # ---- final dense matmul (out^T layout) -----------------------
            xk_srcs = [x0sb[:, :], x1s[0][:, :], x2s[0][:, :],
                       x1s[1][:, :], x2s[1][:, :]]
            for t in range(ntiles):
                acf = psumf.tile([128, fch, 128], F32, tag="of", name="acf")
                xts = []
                for k in range(Km):
                    xt = xkp.tile([128, fch, 128], BF16, tag="xk", name="xt")
                    nc.gpsimd.dma_gather(
                        xt[:], xk_srcs[k], seq_t[:, t * 8:(t + 1) * 8],
                        128, 128, F, transpose=True)
                    xts.append(xt)
                for c in range(fch):
                    for k in range(Km):
                        nc.tensor.matmul(
                            acf[:, c, :], wblk_t[:, k * 128:(k + 1) * 128],
                            xts[k][:, c, :],
                            start=(k == 0), stop=(k == Km - 1))
                for c in range(fch):
                    ob = outp.tile([128, 128], F32, tag="ob", name="ob")
                    nc.vector.tensor_scalar_add(ob[:], acf[:, c, :], bias_t[:])
                    nc.sync.dma_start(outT[c, :, t * 128:(t + 1) * 128], ob[:])

    nc.compile()
    return nc


def _run(x, graphs, weight, biases, ncores=8, sim=False, profile_dir=None):
    """x: (B, N, D) f32; graphs: [(rows, cols, vals)] * 2."""
    B, N, D = x.shape
    F = B * D
    Km = 5
    O = weight.shape[1]
    assert D == 64 and O == 64 and weight.shape[0] == D * Km
    nloc = N // ncores
    ntiles = math.ceil(nloc / 128)
    nlocp = ntiles * 128
    half = N // 2
    fch = F // 128

    # ---- host marshalling ------------------------------------------------
    x0 = np.ascontiguousarray(
        np.transpose(np.asarray(x, np.float32), (1, 0, 2)).reshape(N, F))
    x0_bf = x0.astype(ml_dtypes.bfloat16)

    preps = [_prep_graph(r, c, v, N, ncores, ntiles) for (r, c, v) in graphs]
    nch = max(1, *(int(math.ceil(p["counts"].max() / 128)) for p in preps))
    import os as _os
    if _os.environ.get("DGC_FORCE_NCH"):
        nch = max(nch, int(_os.environ["DGC_FORCE_NCH"]))
    packed = [_pad_graph(p, nch, ncores, ntiles) for p in preps]

    # weight blocks: Wblk[k][(b2,d), (b2',o)] = blockdiag(Wk, Wk)
    wk = np.asarray(weight, np.float32).reshape(D, Km, O)   # [d, k, o]
    wblk = np.zeros((128, Km * 128), np.float32)
    for k in range(Km):
        for b2 in range(2):
            wblk[b2 * 64:(b2 + 1) * 64, k * 128 + b2 * 64: k * 128 + (b2 + 1) * 64] = \
                wk[:, k, :]
    wblk_bf = wblk.astype(ml_dtypes.bfloat16)

    bias_col = np.tile(np.asarray(biases, np.float32), 2).reshape(128, 1)

    seq = np.arange(nlocp, dtype=np.int64).reshape(ntiles, 128)
    seq_packed = _pack_gather_idx(seq)

    in_maps = []
    for m in range(ncores):
        sl = slice(m * nloc, (m + 1) * nloc)
        x0sb = np.zeros((nlocp, F), ml_dtypes.bfloat16)
        x0sb[:nloc] = x0_bf[sl]
        im = {
            "x0h0": x0_bf[:half], "x0h1": x0_bf[half:],
            "x0sb": x0sb, "x0sf": x0[sl],
            "wblk": wblk_bf, "bias": bias_col, "seqidx": seq_packed,
        }
        for g in range(2):
            idx_t, dl_t, vl_t = packed[g]
            im[f"idx{g}"] = idx_t[m]
            im[f"dl{g}"] = dl_t[m]
            im[f"vl{g}"] = vl_t[m]
        in_maps.append(im)

    nc = _build_program(N, B, D, nch, ncores)

    if sim:
        from concourse.bass_interp import MultiCoreSim
        msim = MultiCoreSim(nc, ncores)
        for m in range(ncores):
            for k, v in in_maps[m].items():
                msim.cores[m].tensor(k)[:] = v
        msim.simulate(check_with_hw=False)
        results = [{"outT": np.asarray(msim.cores[m].mem_tensor("outT"))}
                   for m in range(ncores)]
    else:
        from concourse import bass2jax
        if profile_dir is not None:
            from trn_agent_boot.trn_boot import _ntff_profile_via_ctypes
            hook = _ntff_profile_via_ctypes("/opt/axon/libaxon_pjrt.so")
            with hook(profile_dir, None):
                results = bass2jax.run_bass_via_pjrt(nc, in_maps, n_cores=ncores)
            _run.last_nc = nc
        else:
            results = bass2jax.run_bass_via_pjrt(nc, in_maps, n_cores=ncores)

    out = np.empty((B, N, O), np.float32)
    for m in range(ncores):
        oT = np.asarray(results[m]["outT"], np.float32).reshape(fch, 128, nlocp)[:, :, :nloc]
        # oT[c, b2*64+o, i] -> out[2c+b2, m*nloc+i, o]
        arr = oT.reshape(fch, 2, O, nloc)
        out[:, m * nloc:(m + 1) * nloc, :] = \
            np.transpose(arr, (0, 1, 3, 2)).reshape(B, nloc, O)
    return out


def kernel(x, rows0, cols0, vals0, rows1, cols1, vals1, weight, biases):
    x = np.asarray(x, np.float32)
    out = _run(
        x,
        [(rows0, cols0, vals0), (rows1, cols1, vals1)],
        np.asarray(weight, np.float32),
        np.asarray(biases, np.float32),
        ncores=8,
        sim=False,
    )
    return out.astype(np.float32)
